# revision 11
# baseline (speedup 1.0000x reference)
"""v1: packed-PV + within-rep proj backfill (measured 227473 ns)."""

import numpy as np

import concourse.bass as bass
import concourse.mybir as mybir
import concourse.tile as tile
from concourse import bacc
from concourse.bass_utils import run_bass_kernel_spmd
from concourse.masks import make_identity

F32 = mybir.dt.float32
F32R = mybir.dt.float32r
BF16 = mybir.dt.bfloat16
AF = mybir.ActivationFunctionType

N = 2048
D = 384
H = 8
DK = 48
NCORES = 8
SCALE = 1.0 / float(np.sqrt(N))

NT = N // 128
NC_Q = 2
CQ = N // NC_Q
DT3 = D // 128

QK_DT = BF16
PT_DT = BF16
V_DT = BF16


def build_nc(reps=1, stages="absepnf"):
    nc = bacc.Bacc(debug=False)
    x = nc.declare_dram_parameter("x", [N, D], F32, isOutput=False).ap()
    w_qkv = nc.declare_dram_parameter("W_qkv", [D, 3 * D], F32, isOutput=False).ap()
    w_o = nc.declare_dram_parameter("W_o", [D, D], F32, isOutput=False).ap()
    b_o = nc.declare_dram_parameter("b_o", [D], F32, isOutput=False).ap()
    out = nc.declare_dram_parameter("out", [N, D], F32, isOutput=True).ap()

    with tile.TileContext(nc) as tc:
        _emit(nc, tc, x, w_qkv, w_o, b_o, out, reps, stages)
    nc.compile()
    return nc


def _emit(nc, tc, x, w_qkv, w_o, b_o, out, reps=1, stages="absepnf"):
    from contextlib import ExitStack

    ctx = ExitStack()
    with ctx:
        persist = ctx.enter_context(tc.tile_pool(name="persist", bufs=1))

        ident = persist.tile([128, 128], F32)
        make_identity(nc, ident)

        wqkv_sb = []
        with tc.tile_pool(name="wstage", bufs=2) as wstage:
            for dc in range(DT3):
                w_stage = wstage.tile([128, 3 * D], F32)
                nc.sync.dma_start(out=w_stage, in_=w_qkv[dc * 128 : (dc + 1) * 128, :])
                w_t = persist.tile([128, 3 * D], BF16, tag=f"wqkv{dc}", name=f"wqkv{dc}")
                nc.vector.tensor_copy(w_t, w_stage)
                wqkv_sb.append(w_t)

        wo_sb = []
        for dc in range(DT3):
            wo_t = persist.tile([128, D], F32R, tag=f"wo{dc}", name=f"wo{dc}")
            nc.sync.dma_start(
                out=wo_t, in_=w_o[dc * 128 : (dc + 1) * 128, :].bitcast(F32R)
            )
            wo_sb.append(wo_t)

        b_bcast = persist.tile([128, D], F32)
        b_src = bass.AP(tensor=b_o.tensor, offset=0, ap=[[0, 128], [1, D]])
        nc.sync.dma_start(out=b_bcast, in_=b_src)

        xT = [
            persist.tile([128, N], BF16, tag=f"xT{dc}", name=f"xT{dc}")
            for dc in range(DT3)
        ]
        q_pack = [
            persist.tile([128, N], QK_DT, tag=f"qp{p}", name=f"qp{p}")
            for p in range(H // 2)
        ]
        k_pack = [
            persist.tile([128, N], QK_DT, tag=f"kp{p}", name=f"kp{p}")
            for p in range(H // 2)
        ]
        v_pack = [
            persist.tile([128, H, 64], V_DT, tag=f"vp{nt}", name=f"vp{nt}")
            for nt in range(NT)
        ]
        attn_dense = [
            [
                persist.tile([128, CQ], F32R, tag=f"ad{c}_{d_}", name=f"ad{c}_{d_}")
                for d_ in range(DT3)
            ]
            for c in range(NC_Q)
        ]

        for _rep in range(reps):
            _emit_pipeline(
                nc, tc, x, out, ident, wqkv_sb, wo_sb, b_bcast,
                xT, q_pack, k_pack, v_pack, attn_dense, stages,
            )


def _emit_pipeline(
    nc, tc, x, out, ident, wqkv_sb, wo_sb, b_bcast,
    xT, q_pack, k_pack, v_pack, attn_dense, stages="absepnf",
):
    from contextlib import ExitStack

    if "a" in stages:
      with (
        tc.tile_pool(name="xload", bufs=6) as xload,
        tc.tile_pool(name="tpsum", bufs=3, space="PSUM") as tpsum,
        tc.tile_pool(name="vpsum", bufs=2, space="PSUM") as vpsum,
      ):
        for nt in range(NT):
            x_t = xload.tile([128, D], F32)
            nc.sync.dma_start(out=x_t, in_=x[nt * 128 : (nt + 1) * 128, :])
            for dc in range(DT3):
                p_t = tpsum.tile([128, 128], F32)
                nc.tensor.transpose(p_t, x_t[:, dc * 128 : (dc + 1) * 128], ident)
                nc.vector.tensor_copy(xT[dc][:, nt * 128 : (nt + 1) * 128], p_t)
            pv = vpsum.tile([128, D], F32, tag="pv")
            for dc in range(DT3):
                nc.tensor.matmul(
                    pv, xT[dc][:, nt * 128 : (nt + 1) * 128],
                    wqkv_sb[dc][:, 2 * D : 3 * D],
                    start=(dc == 0), stop=(dc == DT3 - 1),
                )
            vp = v_pack[nt]
            nc.gpsimd.memset(vp, 1.0)
            pv_h = pv.rearrange("p (h c) -> p h c", c=DK)
            nc.vector.tensor_copy(vp[:, :, 0:DK], pv_h)

    stream_ctx = ExitStack()
    with stream_ctx:
        if "b" in stages:
            projpsum = stream_ctx.enter_context(
                tc.tile_pool(name="projpsum", bufs=2, space="PSUM")
            )

        def emit_proj(pair, qk, c4):
            hA, hB = 2 * pair, 2 * pair + 1
            dest = q_pack[pair] if qk == 0 else k_pack[pair]
            qoff = 0 if qk == 0 else D
            cs = slice(c4 * 512, (c4 + 1) * 512)
            pp = projpsum.tile([128, 512], F32, tag="pp")
            for dc in range(DT3):
                nc.tensor.matmul(
                    pp[0:48, :],
                    wqkv_sb[dc][:, qoff + hA * DK : qoff + hA * DK + DK],
                    xT[dc][:, cs],
                    start=(dc == 0), stop=(dc == DT3 - 1),
                )
                nc.tensor.matmul(
                    pp[64:112, :],
                    wqkv_sb[dc][:, qoff + hB * DK : qoff + hB * DK + DK],
                    xT[dc][:, cs],
                    start=(dc == 0), stop=(dc == DT3 - 1),
                    tile_position=(0, 64),
                )
            nc.vector.tensor_copy(dest[0:112, cs], pp[0:112, :])

        if "b" in stages:
            prologue = [(0, 0, 0)] + [(0, 1, c4) for c4 in range(4)]
            for pr in prologue:
                emit_proj(*pr)

            items = []
            for p in range(H // 2):
                for c4 in range(4):
                    if (p, 0, c4) not in prologue:
                        items.append((p * 64 + c4 * 16, (p, 0, c4)))
                    if (p, 1, c4) not in prologue:
                        items.append((p * 64 + 4 * c4, (p, 1, c4)))
            items.sort()
            backfill = {}
            next_free = 0
            for deadline, args in items:
                w = max(next_free, deadline - 12)
                next_free = w + 1
                backfill.setdefault(w, []).append(args)
        else:
            backfill = {}

        if "s" in stages:
          with (
            tc.tile_pool(name="spsum", bufs=2, space="PSUM") as spsum,
            tc.tile_pool(name="opsum", bufs=2, space="PSUM") as opsum,
            tc.tile_pool(name="ptpool", bufs=4) as ptpool,
            tc.tile_pool(name="zpool", bufs=2) as zpool,
            tc.tile_pool(name="stpool", bufs=2) as stpool,
          ):
            for pair in range(H // 2):
                hA, hB = 2 * pair, 2 * pair + 1
                qp, kp = q_pack[pair], k_pack[pair]
                for c5 in range(N // 512):
                    w_base = pair * 64 + c5 * 16
                    cqs = slice(c5 * 512, (c5 + 1) * 512)
                    oAB = opsum.tile([128, 512], F32, tag="oAB")
                    pend = None

                    def emit_pv(pend):
                        t, ptAB = pend
                        nc.tensor.matmul(
                            oAB[0:64, :], v_pack[t][:, hA, :], ptAB[:, 0:512],
                            start=(t == 0), stop=(t == NT - 1),
                        )
                        nc.tensor.matmul(
                            oAB[64:128, :], v_pack[t][:, hB, :], ptAB[:, 512:1024],
                            start=(t == 0), stop=(t == NT - 1),
                            tile_position=(0, 64),
                        )

                    for t in range(NT):
                        for args in backfill.pop(w_base + t, ()):
                            emit_proj(*args)
                        ts_ = slice(t * 128, (t + 1) * 128)
                        sAB = spsum.tile([128, 1024], F32, tag="sAB")
                        nc.tensor.matmul(
                            sAB[:, 0:512], kp[0:48, ts_], qp[0:48, cqs],
                            start=True, stop=True,
                        )
                        nc.tensor.matmul(
                            sAB[:, 512:1024], kp[64:112, ts_], qp[64:112, cqs],
                            start=True, stop=True,
                        )
                        if "e" not in stages:
                            continue
                        ptAB = ptpool.tile([128, 1024], PT_DT, tag="ptAB")
                        nc.scalar.activation(ptAB, sAB, AF.Exp, scale=SCALE)
                        if "p" not in stages:
                            continue
                        if pend is not None:
                            emit_pv(pend)
                        pend = (t, ptAB)
                    if pend is not None:
                        emit_pv(pend)
                        pend = None
                    if "n" not in stages:
                        continue
                    zr = zpool.tile([128, 512], F32, tag="zr")
                    nc.vector.reciprocal(zr[32:64, :], oAB[32:64, :])
                    nc.vector.reciprocal(zr[96:128, :], oAB[96:128, :])
                    zsA = zpool.tile([48, 512], F32, tag="zsA")
                    for r in range(3):
                        nc.sync.dma_start(out=zsA[16 * r : 16 * r + 16, :],
                                          in_=zr[48:64, :])
                    stA = stpool.tile([48, 512], F32R, tag="stA")
                    nc.vector.tensor_mul(stA, oAB[0:48, :], zsA)
                    zsB = zpool.tile([48, 512], F32, tag="zsB")
                    for r in range(3):
                        nc.sync.dma_start(out=zsB[16 * r : 16 * r + 16, :],
                                          in_=zr[112:128, :])
                    stB = stpool.tile([48, 512], F32R, tag="stB")
                    nc.vector.tensor_mul(stB, oAB[64:112, :], zsB)

                    c = (c5 * 512) // CQ
                    col = (c5 * 512) % CQ
                    for h, src in ((hA, stA), (hB, stB)):
                        r0 = h * DK
                        d0, o0 = r0 // 128, r0 % 128
                        n0 = min(48, 128 - o0)
                        nc.sync.dma_start(
                            out=attn_dense[c][d0][o0 : o0 + n0, col : col + 512],
                            in_=src[0:n0, :],
                        )
                        if n0 < 48:
                            nc.sync.dma_start(
                                out=attn_dense[c][d0 + 1][0 : 48 - n0, col : col + 512],
                                in_=src[n0:48, :],
                            )

    if "f" in stages:
      with (
        tc.tile_pool(name="fpsum", bufs=2, space="PSUM") as fpsum,
        tc.tile_pool(name="fout", bufs=3) as fout,
      ):
        for nt in range(NT):
            c = (nt * 128) // CQ
            col = (nt * 128) % CQ
            cslice = slice(col, col + 128)
            pf = fpsum.tile([128, D], F32, tag="pf")
            for dc in range(DT3):
                nc.tensor.matmul(
                    pf,
                    attn_dense[c][dc][:, cslice],
                    wo_sb[dc],
                    start=(dc == 0),
                    stop=(dc == DT3 - 1),
                )
            o_t = fout.tile([128, D], F32)
            nc.vector.tensor_add(o_t, pf, b_bcast)
            nc.sync.dma_start(out=out[nt * 128 : (nt + 1) * 128, :], in_=o_t)


_NC_CACHE = None


def _get_nc():
    global _NC_CACHE
    if _NC_CACHE is None:
        _NC_CACHE = build_nc()
    return _NC_CACHE


def kernel(x, W_qkv, W_o, b_o):
    x = np.asarray(x, dtype=np.float32)
    W_qkv = np.ascontiguousarray(np.asarray(W_qkv, dtype=np.float32))
    W_o = np.ascontiguousarray(np.asarray(W_o, dtype=np.float32))
    b_o = np.ascontiguousarray(np.asarray(b_o, dtype=np.float32))
    b, p, n, d = x.shape
    assert (b, p, n, d) == (NCORES, 1, N, D), x.shape

    nc = _get_nc()
    in_maps = [
        {
            "x": np.ascontiguousarray(x[i, 0]),
            "W_qkv": W_qkv,
            "W_o": W_o,
            "b_o": b_o,
        }
        for i in range(NCORES)
    ]
    res = run_bass_kernel_spmd(nc, in_maps, core_ids=list(range(NCORES)))
    outs = np.stack([res.results[i]["out"] for i in range(NCORES)])
    return outs[:, None].astype(np.float32)


# revision 14
# speedup vs baseline: 2.3268x; 2.3268x over previous
"""Multi-head attention kernel for Trainium2, distributed over 8 NeuronCores.

Problem: x[8,1,2048,384] @ W_qkv[384,1152] -> 8-head attention (dk=48,
softmax scale 1/sqrt(2048)) -> @ W_o[384,384] + b_o.

Sharding: batch (b=8) data-parallel, one batch element per core. No
collectives.

The wall-clock floor is ScalarE: softmax needs exp of h*n^2 = 33.5M
elements/core, and ScalarE (the only exp engine; GPSIMD has no PSUM port,
DVE has no exp) streams 128 lanes @ 1.2 GHz => ~218us pure datapath. The
kernel's whole job is to keep the 256 [128,1024] exp ops back-to-back:

  1. Prep: xT via PE transpose (fp32, exact), stored bf16; v projection
     packed per n-tile as v_pack[t][128, h, 64] = [v48 | ones16] so the PV
     matmul also emits the softmax denominator Z (rows 48:64 of each head's
     output strip). q/k projections computed TRANSPOSED ([dk, n] layout) in
     bf16, two heads packed per 128-partition tile (head A rows 0:48, head
     B rows 64:112, col-packed via tile_position); PSUM->SBUF copies on
     VectorE so ScalarE's FIFO only ever holds exps.
  2. Attention, pair-outer: per (head-pair, c5 chunk of 512 q, n_k tile t
     of 16): the two heads' S^T matmuls write ONE 2-bank PSUM tile (cols
     0:512 = A, 512:1024 = B) so they sit adjacent in the in-order PE
     queue and run CONCURRENTLY on disjoint row strips; one [128, 1024]
     exp per tile on ScalarE straight from PSUM (1/sqrt(n) scale folded
     into the activation affine), P^T bf16. PV lags one tile so its
     exp-dependency is always satisfied when it reaches the PE queue head;
     both heads' PV accumulate into ONE 1-bank PSUM tile oAB (head A rows
     0:64 incl Z, head B rows 64:128 via tile_position) -- the partition
     packing halves opsum to 2 banks, which lets the projection PSUM pool
     (2 banks) coexist with the attention stream (spsum 4 banks).
  3. Projections for pairs 1..3 are deadline-scheduled into earlier pairs'
     attention windows (one ~0.6us PE group per window) so the PE's
     in-order queue never makes the exp stream wait on projection work;
     the Tile scheduler overlaps each rep's prep/tail with its neighbors.
  4. Normalization: DVE reciprocal of the 16 Z rows (over a 32-aligned
     partition superset; the extra rows are junk no one reads), partition-
     replicate x3 via DMA, DVE multiply -> dense attn^T [384, n_q] f32r
     tiles via repack DMA.
  5. fc_o consumes dense attn^T as lhsT (3 matmuls at full K=128) ->
     output lands in NATURAL [n, d] layout; bias added on VectorE against
     a DMA-broadcast b_o.

Measured on TRN2 (rep-differenced, quiet device): ~227k ns/core, vs 353k
for the previous version; max rel err ~2.5e-3 vs the fp32 reference (bf16
operand rounding).
"""

import numpy as np

import concourse.bass as bass
import concourse.mybir as mybir
import concourse.tile as tile
from concourse import bacc
from concourse.bass_utils import run_bass_kernel_spmd
from concourse.masks import make_identity

F32 = mybir.dt.float32
F32R = mybir.dt.float32r
BF16 = mybir.dt.bfloat16
AF = mybir.ActivationFunctionType

N = 2048
D = 384
H = 8
DK = 48
NCORES = 8
SCALE = 1.0 / float(np.sqrt(N))

NT = N // 128
NC_Q = 2
CQ = N // NC_Q
DT3 = D // 128

QK_DT = BF16
PT_DT = BF16
V_DT = BF16


def build_nc(reps=1, stages="absepnf"):
    nc = bacc.Bacc(debug=False)
    x = nc.declare_dram_parameter("x", [N, D], F32, isOutput=False).ap()
    w_qkv = nc.declare_dram_parameter("W_qkv", [D, 3 * D], F32, isOutput=False).ap()
    w_o = nc.declare_dram_parameter("W_o", [D, D], F32, isOutput=False).ap()
    b_o = nc.declare_dram_parameter("b_o", [D], F32, isOutput=False).ap()
    out = nc.declare_dram_parameter("out", [N, D], F32, isOutput=True).ap()

    with tile.TileContext(nc) as tc:
        _emit(nc, tc, x, w_qkv, w_o, b_o, out, reps, stages)
    nc.compile()
    return nc


def _emit(nc, tc, x, w_qkv, w_o, b_o, out, reps=1, stages="absepnf"):
    from contextlib import ExitStack

    ctx = ExitStack()
    with ctx:
        persist = ctx.enter_context(tc.tile_pool(name="persist", bufs=1))

        ident = persist.tile([128, 128], F32)
        make_identity(nc, ident)

        wqkv_sb = []
        with tc.tile_pool(name="wstage", bufs=2) as wstage:
            for dc in range(DT3):
                w_stage = wstage.tile([128, 3 * D], F32)
                nc.sync.dma_start(out=w_stage, in_=w_qkv[dc * 128 : (dc + 1) * 128, :])
                w_t = persist.tile([128, 3 * D], BF16, tag=f"wqkv{dc}", name=f"wqkv{dc}")
                nc.vector.tensor_copy(w_t, w_stage)
                wqkv_sb.append(w_t)

        wo_sb = []
        for dc in range(DT3):
            wo_t = persist.tile([128, D], F32R, tag=f"wo{dc}", name=f"wo{dc}")
            nc.sync.dma_start(
                out=wo_t, in_=w_o[dc * 128 : (dc + 1) * 128, :].bitcast(F32R)
            )
            wo_sb.append(wo_t)

        b_bcast = persist.tile([128, D], F32)
        b_src = bass.AP(tensor=b_o.tensor, offset=0, ap=[[0, 128], [1, D]])
        nc.sync.dma_start(out=b_bcast, in_=b_src)

        xT = [
            persist.tile([128, N], BF16, tag=f"xT{dc}", name=f"xT{dc}")
            for dc in range(DT3)
        ]
        q_pack = [
            persist.tile([128, N], QK_DT, tag=f"qp{p}", name=f"qp{p}")
            for p in range(H // 2)
        ]
        k_pack = [
            persist.tile([128, N], QK_DT, tag=f"kp{p}", name=f"kp{p}")
            for p in range(H // 2)
        ]
        v_pack = [
            persist.tile([128, H, 64], V_DT, tag=f"vp{nt}", name=f"vp{nt}")
            for nt in range(NT)
        ]
        attn_dense = [
            [
                persist.tile([128, CQ], F32R, tag=f"ad{c}_{d_}", name=f"ad{c}_{d_}")
                for d_ in range(DT3)
            ]
            for c in range(NC_Q)
        ]

        for _rep in range(reps):
            _emit_pipeline(
                nc, tc, x, out, ident, wqkv_sb, wo_sb, b_bcast,
                xT, q_pack, k_pack, v_pack, attn_dense, stages,
            )


def _emit_pipeline(
    nc, tc, x, out, ident, wqkv_sb, wo_sb, b_bcast,
    xT, q_pack, k_pack, v_pack, attn_dense, stages="absepnf",
):
    from contextlib import ExitStack

    if "a" in stages:
      with (
        tc.tile_pool(name="xload", bufs=6) as xload,
        tc.tile_pool(name="tpsum", bufs=3, space="PSUM") as tpsum,
        tc.tile_pool(name="vpsum", bufs=2, space="PSUM") as vpsum,
      ):
        for nt in range(NT):
            x_t = xload.tile([128, D], F32)
            nc.sync.dma_start(out=x_t, in_=x[nt * 128 : (nt + 1) * 128, :])
            for dc in range(DT3):
                p_t = tpsum.tile([128, 128], F32)
                nc.tensor.transpose(p_t, x_t[:, dc * 128 : (dc + 1) * 128], ident)
                nc.vector.tensor_copy(xT[dc][:, nt * 128 : (nt + 1) * 128], p_t)
            pv = vpsum.tile([128, D], F32, tag="pv")
            for dc in range(DT3):
                nc.tensor.matmul(
                    pv, xT[dc][:, nt * 128 : (nt + 1) * 128],
                    wqkv_sb[dc][:, 2 * D : 3 * D],
                    start=(dc == 0), stop=(dc == DT3 - 1),
                )
            vp = v_pack[nt]
            nc.gpsimd.memset(vp, 1.0)
            pv_h = pv.rearrange("p (h c) -> p h c", c=DK)
            nc.vector.tensor_copy(vp[:, :, 0:DK], pv_h)

    stream_ctx = ExitStack()
    with stream_ctx:
        if "b" in stages:
            projpsum = stream_ctx.enter_context(
                tc.tile_pool(name="projpsum", bufs=2, space="PSUM")
            )

        def emit_proj(pair, qk, c4):
            hA, hB = 2 * pair, 2 * pair + 1
            dest = q_pack[pair] if qk == 0 else k_pack[pair]
            qoff = 0 if qk == 0 else D
            cs = slice(c4 * 512, (c4 + 1) * 512)
            pp = projpsum.tile([128, 512], F32, tag="pp")
            for dc in range(DT3):
                nc.tensor.matmul(
                    pp[0:48, :],
                    wqkv_sb[dc][:, qoff + hA * DK : qoff + hA * DK + DK],
                    xT[dc][:, cs],
                    start=(dc == 0), stop=(dc == DT3 - 1),
                )
                nc.tensor.matmul(
                    pp[64:112, :],
                    wqkv_sb[dc][:, qoff + hB * DK : qoff + hB * DK + DK],
                    xT[dc][:, cs],
                    start=(dc == 0), stop=(dc == DT3 - 1),
                    tile_position=(0, 64),
                )
            nc.vector.tensor_copy(dest[0:112, cs], pp[0:112, :])

        if "b" in stages:
            prologue = [(0, 0, 0)] + [(0, 1, c4) for c4 in range(4)]
            for pr in prologue:
                emit_proj(*pr)

            items = []
            for p in range(H // 2):
                for c4 in range(4):
                    if (p, 0, c4) not in prologue:
                        items.append((p * 64 + c4 * 16, (p, 0, c4)))
                    if (p, 1, c4) not in prologue:
                        items.append((p * 64 + 4 * c4, (p, 1, c4)))
            items.sort()
            backfill = {}
            next_free = 0
            for deadline, args in items:
                w = max(next_free, deadline - 12)
                next_free = w + 1
                backfill.setdefault(w, []).append(args)
        else:
            backfill = {}

        if "s" in stages:
          with (
            tc.tile_pool(name="spsum", bufs=2, space="PSUM") as spsum,
            tc.tile_pool(name="opsum", bufs=2, space="PSUM") as opsum,
            tc.tile_pool(name="ptpool", bufs=4) as ptpool,
            tc.tile_pool(name="zpool", bufs=2) as zpool,
            tc.tile_pool(name="stpool", bufs=2) as stpool,
          ):
            for pair in range(H // 2):
                hA, hB = 2 * pair, 2 * pair + 1
                qp, kp = q_pack[pair], k_pack[pair]
                for c5 in range(N // 512):
                    w_base = pair * 64 + c5 * 16
                    cqs = slice(c5 * 512, (c5 + 1) * 512)
                    oAB = opsum.tile([128, 512], F32, tag="oAB")
                    pend = None

                    def emit_pv(pend):
                        t, ptAB = pend
                        nc.tensor.matmul(
                            oAB[0:64, :], v_pack[t][:, hA, :], ptAB[:, 0:512],
                            start=(t == 0), stop=(t == NT - 1),
                        )
                        nc.tensor.matmul(
                            oAB[64:128, :], v_pack[t][:, hB, :], ptAB[:, 512:1024],
                            start=(t == 0), stop=(t == NT - 1),
                            tile_position=(0, 64),
                        )

                    for t in range(NT):
                        for args in backfill.pop(w_base + t, ()):
                            emit_proj(*args)
                        ts_ = slice(t * 128, (t + 1) * 128)
                        sAB = spsum.tile([128, 1024], F32, tag="sAB")
                        nc.tensor.matmul(
                            sAB[:, 0:512], kp[0:48, ts_], qp[0:48, cqs],
                            start=True, stop=True,
                        )
                        nc.tensor.matmul(
                            sAB[:, 512:1024], kp[64:112, ts_], qp[64:112, cqs],
                            start=True, stop=True,
                        )
                        if "e" not in stages:
                            continue
                        ptAB = ptpool.tile([128, 1024], PT_DT, tag="ptAB")
                        nc.scalar.activation(ptAB, sAB, AF.Exp, scale=SCALE)
                        if "p" not in stages:
                            continue
                        if pend is not None:
                            emit_pv(pend)
                        pend = (t, ptAB)
                    if pend is not None:
                        emit_pv(pend)
                        pend = None
                    if "n" not in stages:
                        continue
                    zr = zpool.tile([128, 512], F32, tag="zr")
                    nc.vector.reciprocal(zr[32:64, :], oAB[32:64, :])
                    nc.vector.reciprocal(zr[96:128, :], oAB[96:128, :])
                    zsA = zpool.tile([48, 512], F32, tag="zsA")
                    for r in range(3):
                        nc.sync.dma_start(out=zsA[16 * r : 16 * r + 16, :],
                                          in_=zr[48:64, :])
                    stA = stpool.tile([48, 512], F32R, tag="stA")
                    nc.vector.tensor_mul(stA, oAB[0:48, :], zsA)
                    zsB = zpool.tile([48, 512], F32, tag="zsB")
                    for r in range(3):
                        nc.sync.dma_start(out=zsB[16 * r : 16 * r + 16, :],
                                          in_=zr[112:128, :])
                    stB = stpool.tile([48, 512], F32R, tag="stB")
                    nc.vector.tensor_mul(stB, oAB[64:112, :], zsB)

                    c = (c5 * 512) // CQ
                    col = (c5 * 512) % CQ
                    for h, src in ((hA, stA), (hB, stB)):
                        r0 = h * DK
                        d0, o0 = r0 // 128, r0 % 128
                        n0 = min(48, 128 - o0)
                        nc.sync.dma_start(
                            out=attn_dense[c][d0][o0 : o0 + n0, col : col + 512],
                            in_=src[0:n0, :],
                        )
                        if n0 < 48:
                            nc.sync.dma_start(
                                out=attn_dense[c][d0 + 1][0 : 48 - n0, col : col + 512],
                                in_=src[n0:48, :],
                            )

    if "f" in stages:
      with (
        tc.tile_pool(name="fpsum", bufs=2, space="PSUM") as fpsum,
        tc.tile_pool(name="fout", bufs=3) as fout,
      ):
        for nt in range(NT):
            c = (nt * 128) // CQ
            col = (nt * 128) % CQ
            cslice = slice(col, col + 128)
            pf = fpsum.tile([128, D], F32, tag="pf")
            for dc in range(DT3):
                nc.tensor.matmul(
                    pf,
                    attn_dense[c][dc][:, cslice],
                    wo_sb[dc],
                    start=(dc == 0),
                    stop=(dc == DT3 - 1),
                )
            o_t = fout.tile([128, D], F32)
            nc.vector.tensor_add(o_t, pf, b_bcast)
            nc.sync.dma_start(out=out[nt * 128 : (nt + 1) * 128, :], in_=o_t)


_NC_CACHE = None


def _get_nc():
    global _NC_CACHE
    if _NC_CACHE is None:
        _NC_CACHE = build_nc()
    return _NC_CACHE


def kernel(x, W_qkv, W_o, b_o):
    x = np.asarray(x, dtype=np.float32)
    W_qkv = np.ascontiguousarray(np.asarray(W_qkv, dtype=np.float32))
    W_o = np.ascontiguousarray(np.asarray(W_o, dtype=np.float32))
    b_o = np.ascontiguousarray(np.asarray(b_o, dtype=np.float32))
    b, p, n, d = x.shape
    assert (b, p, n, d) == (NCORES, 1, N, D), x.shape

    nc = _get_nc()
    in_maps = [
        {
            "x": np.ascontiguousarray(x[i, 0]),
            "W_qkv": W_qkv,
            "W_o": W_o,
            "b_o": b_o,
        }
        for i in range(NCORES)
    ]
    res = run_bass_kernel_spmd(nc, in_maps, core_ids=list(range(NCORES)))
    outs = np.stack([res.results[i]["out"] for i in range(NCORES)])
    return outs[:, None].astype(np.float32)
